# revision 30
# baseline (speedup 1.0000x reference)
"""Multi-head attention, tensor-parallel over heads x data-parallel over batch.

8 NeuronCores: core c handles batch b=c//2, head-group g=c%2 (8 heads, 512 chans).
Each core computes its head-group's attention + partial output projection;
the two partials per batch are summed on the host (row-parallel Wo unshard).

Design (vs the fp32r baseline: 234us -> ~163us):
  - every matmul operand is bf16 (halves DMA + LDWEIGHTS; PSUM stays fp32;
    rel err 7e-3 vs the 2e-2 gate)
  - softmax 1/sum on VectorE via reciprocal_approx_fast, so ScalarE runs
    ONLY Exp (one ACT table set, no FIFO clog from the old ln/exp norm).
    recip silently misreads partition-offset APs, hence the [ones|v]
    stationary packing that puts the sums on partitions 0:64.
  - fp16 output with paired [128,1024] staging (2KB DMA lines), halves the
    output-drain tail
  - only k0/q0 ic0 projection chains run upfront; every other chain (v,
    qk of later pairs, out-proj of the ih0 token half) is interleaved into
    the attention jb-loops via the fill schedule, so TensorE streams
    gap-free behind the ~40us DMA-bandwidth-bound input window
  - one-step block lookahead: the next block's scores+exp are emitted
    before this block's AV-drain + norm (no engine idles at boundaries)
  - input DMA in ~128KB pieces over the sync+gpsimd queues only (each
    dma_start rides ONE ~22GB/s ring; scalar-queue issues would gate the
    first exp); three waves ordered by first use

Per-core dataflow:
  qT/kT = W.T @ xT        [chan, tok] transposed projections
  v     = xkv @ Wv        [tok, chan] natural projection, stored per head
                          pair as [ones|v_even|ones|v_odd] (64 cols each)
  scoresT[j,i] = k.q      row-group packed pairs -> one [128,1024] psum
  expT  = exp(s*scale + maskbias)   one ScalarE op per (pair, ih, jb), bf16 out
  AV:   lhsT = [ones|v_h] (M=128) -> po rows = [sums replicated | o_h]
  norm: rb = recip(sums) on VectorE, oT = po * rb on VectorE (bf16 out)
  out   = oT.T @ Wo       partial output projection
"""

import numpy as np

import concourse.bacc as bacc
import concourse.mybir as mybir
import concourse.tile as tile
from concourse import bass_utils

B = 4
T = 1024          # tokens (N = L)
D = 1024          # model dim
CH = 64           # channels per head
G = 512           # channels per head-group (8 heads)
SCALE = CH ** -0.5
NEG = -30000.0    # mask bias (exp(x + NEG) == 0)
F32 = mybir.dt.float32
F16 = mybir.dt.float16
BF16 = mybir.dt.bfloat16
BF_NP = mybir.dt.np(mybir.dt.bfloat16)
F16_NP = mybir.dt.np(mybir.dt.float16)

N_CORES = 8
KB = 8            # 128-row contraction blocks over D
TB = 8            # 128-token blocks
PAIRS = 4         # head pairs per core
VW = 256          # v-tile columns per head pair: [ones|v_even|ones|v_odd]
AV_LAG = 2        # software-pipeline depth: AV trails scores/exp by this many jb

LAST_RESULTS = None
_CACHE = {}


def _emit(tc):
    nc = tc.nc
    xqT = nc.dram_tensor("xqT", [D, T], BF16, kind="ExternalInput").ap()
    xkvT = nc.dram_tensor("xkvT", [D, T], BF16, kind="ExternalInput").ap()
    wq = nc.dram_tensor("wq", [D, G], BF16, kind="ExternalInput").ap()
    wk = nc.dram_tensor("wk", [D, G], BF16, kind="ExternalInput").ap()
    wv = nc.dram_tensor("wv", [D, G], BF16, kind="ExternalInput").ap()
    wo = nc.dram_tensor("wo", [G, D], BF16, kind="ExternalInput").ap()
    mb = nc.dram_tensor("mb", [128, TB], F32, kind="ExternalInput").ap()
    vones = nc.dram_tensor("vones", [128, PAIRS * 2 * CH], BF16,
                           kind="ExternalInput").ap()
    out = nc.dram_tensor("out", [T, D], F16, kind="ExternalOutput").ap()

    Exp = mybir.ActivationFunctionType.Exp

    with (
        tc.tile_pool(name="wpool", bufs=1) as wpool,
        tc.tile_pool(name="xpool", bufs=1) as xpool,
        tc.tile_pool(name="apool", bufs=1) as apool,
        tc.tile_pool(name="epool", bufs=1) as epool,
        tc.tile_pool(name="npool", bufs=1) as npool,
        tc.tile_pool(name="opool", bufs=4) as opool,
        tc.tile_pool(name="psum", bufs=1, space="PSUM") as psum,
    ):
        # ---------------- input DMA ----------------
        # One dma_start rides ONE queue (~23 GB/s), so transfers are split
        # per 128KB piece and spread over the sync/scalar/gpsimd queues in
        # first-use order. Issue cost is ~0.6-0.9us per dma_start, so each
        # queue carries only what it must before its compute begins.
        mask_t = wpool.tile([128, TB], F32, name="mask_t", tag="mask")

        xkv_b = xpool.tile([128, KB * T], BF16, name="xkv", tag="xkv")
        xq_b = xpool.tile([128, KB * T], BF16, name="xq", tag="xq")
        xkv_src = xkvT.rearrange("(kb p) t -> p kb t", p=128)
        xq_src = xqT.rearrange("(kb p) t -> p kb t", p=128)
        xkv_dst = xkv_b.rearrange("p (kb t) -> p kb t", t=T)
        xq_dst = xq_b.rearrange("p (kb t) -> p kb t", t=T)
        xkv_t = [xkv_b[:, k * T:(k + 1) * T] for k in range(KB)]
        xq_t = [xq_b[:, k * T:(k + 1) * T] for k in range(KB)]

        wv_b = wpool.tile([128, KB * G], BF16, name="wv", tag="wv")
        wq_b = wpool.tile([128, KB * G], BF16, name="wq", tag="wq")
        wk_b = wpool.tile([128, KB * G], BF16, name="wk", tag="wk")
        wv_t = [wv_b[:, k * G:(k + 1) * G] for k in range(KB)]
        wq_t = [wq_b[:, k * G:(k + 1) * G] for k in range(KB)]
        wk_t = [wk_b[:, k * G:(k + 1) * G] for k in range(KB)]
        wo_b = wpool.tile([128, PAIRS * D], BF16, name="wo", tag="wo")
        wo_t = [wo_b[:, m * D:(m + 1) * D] for m in range(PAIRS)]

        v_t = [apool.tile([128, PAIRS * VW], BF16, name=f"v{tb}",
                          tag=f"v{tb}") for tb in range(TB)]
        vo_src = vones.rearrange("p (pb f c) -> p pb f c", f=2, c=CH)
        wdst = {id(wv_b): wv_b.rearrange("p (kb g) -> p kb g", g=G),
                id(wq_b): wq_b.rearrange("p (kb g) -> p kb g", g=G),
                id(wk_b): wk_b.rearrange("p (kb g) -> p kb g", g=G)}
        wsrc = {id(wv_b): wv.rearrange("(kb p) g -> p kb g", p=128),
                id(wq_b): wq.rearrange("(kb p) g -> p kb g", p=128),
                id(wk_b): wk.rearrange("(kb p) g -> p kb g", p=128)}
        wo_dst = wo_b.rearrange("p (m d) -> p m d", d=D)
        wo_src2 = wo.rearrange("(m p) d -> p m d", p=128)

        # Waves balanced across the sync/gpsimd queues ONLY -- a DMA issue
        # on the scalar queue would gate the first exp behind the issue
        # stream. Many ~128KB pieces: each dma_start rides one DMA ring, so
        # small pieces across many rings is what parallelizes the head.
        # wave 1: what k0ic0/q0ic0 + scores(p0,ih0,jb0-3) need
        nc.sync.dma_start(mask_t[:], mb[:])
        qq = (nc.sync, nc.gpsimd)
        # k0ic0's very first matmul needs only wk[0][:,0:128] and the token
        # half of xkv[0]: tiny first pieces land in ~5-9us instead of the
        # 12-17us a full 256KB single-ring piece takes, starting the PE (and
        # its HAM warmup) that much earlier
        nc.gpsimd.dma_start(wdst[id(wk_b)][:, 0, 0:128],
                            wsrc[id(wk_b)][:, 0, 0:128])
        nc.sync.dma_start(xkv_dst[:, 0, 0:512], xkv_src[:, 0, 0:512])
        nc.gpsimd.dma_start(wdst[id(wk_b)][:, 0, 128:512],
                            wsrc[id(wk_b)][:, 0, 128:512])
        nc.sync.dma_start(xkv_dst[:, 0, 512:1024], xkv_src[:, 0, 512:1024])
        # xkv 1-3 as halves too: the k0ic0 chain streams behind them from
        # ~11us instead of stalling 8us on a full-row single-ring piece
        for k in (1, 2, 3):
            nc.sync.dma_start(xkv_dst[:, k, 0:512], xkv_src[:, k, 0:512])
            nc.gpsimd.dma_start(xkv_dst[:, k, 512:1024],
                                xkv_src[:, k, 512:1024])
        # full-row x chunks: 2KB lines run at 21.9 GB/s/ring vs 16.4 for 1KB
        for k in range(4, KB):
            qq[k % 2].dma_start(xkv_dst[:, k, :], xkv_src[:, k, :])
        for k in range(1, KB):
            qq[(k + 1) % 2].dma_start(wdst[id(wk_b)][:, k, :],
                                      wsrc[id(wk_b)][:, k, :])
        # wv in wave 1: the v chains run upfront, in the head's otherwise
        # DMA-paced Tensor idle, which empties ~15us of matmuls out of the
        # (Tensor-bound) attention window
        for k in range(KB):
            qq[k % 2].dma_start(wdst[id(wv_b)][:, k, :], wsrc[id(wv_b)][:, k, :])
        for k in range(KB):
            qq[(k + 1) % 2].dma_start(xq_dst[:, k, :], xq_src[:, k, :])
        for k in range(KB):
            qq[k % 2].dma_start(wdst[id(wq_b)][:, k, :],
                                wsrc[id(wq_b)][:, k, :])
        # wave 2
        for m in range(PAIRS):
            qq[m % 2].dma_start(wo_dst[:, m, :], wo_src2[:, m, :])
        # ones blocks via GpSimd memset -- the DMA pattern moved 0.5MB in
        # 128B packets (7 GB/s) through the head's critical DMA window
        for tb in range(TB):
            v2c = v_t[tb].rearrange("p (pb f c) -> p pb f c", f=2, c=128)
            nc.gpsimd.memset(v2c[:, :, :, 0:CH], 1.0)

        # ---------------- v = xkv @ Wv (natural layout) ----------------
        def v_chain(tb):
            ps = psum.tile([128, 512], F32, name="ps_acc", tag="acc", bufs=2)
            for k in range(KB):
                nc.tensor.matmul(
                    ps[:],
                    xkv_t[k][:, tb * 128:(tb + 1) * 128],
                    wv_t[k][:],
                    start=(k == 0),
                    stop=(k == KB - 1),
                )
            v4 = v_t[tb].rearrange("p (pb four c) -> p pb four c",
                                   four=4, c=CH)
            ps3 = ps.rearrange("p (pb two c) -> p pb two c", two=2, c=CH)
            nc.vector.tensor_copy(v4[:, :, 1:2, :], ps3[:, :, 0:1, :])
            nc.vector.tensor_copy(v4[:, :, 3:4, :], ps3[:, :, 1:2, :])

        # (v chains are emitted between k0ic0 and q0ic0 below)

        # ---------------- qT/kT projection chains ----------------
        qT_t = [apool.tile([128, T], BF16, name=f"qT{m}", tag=f"qT{m}")
                for m in range(PAIRS)]
        kT_t = [apool.tile([128, T], BF16, name=f"kT{m}", tag=f"kT{m}")
                for m in range(PAIRS)]

        def proj_chain(p, which, ic):
            src, w_t, dst = ((xq_t, wq_t, qT_t), (xkv_t, wk_t, kT_t))[which]
            csl = slice(ic * 512, (ic + 1) * 512)
            ps = psum.tile([128, 512], F32, name="ps_acc", tag="acc", bufs=2)
            for k in range(KB):
                nc.tensor.matmul(
                    ps[:],
                    w_t[k][:, p * 128:(p + 1) * 128],
                    src[k][:, csl],
                    start=(k == 0),
                    stop=(k == KB - 1),
                )
            nc.vector.tensor_copy(dst[p][:, csl], ps[:])

        # out-projection chain; tb 0..3 read only the ih=0 half of oT, so
        # they can interleave into pair 3's attention
        oq_cycle = [None, None, None]  # filled below (queues)

        ot_full = {}

        def out_chain(tb, ncx, oT_t, trailing=False):
            tsl = slice(tb * 128, (tb + 1) * 128)
            nsl = slice(ncx * 512, (ncx + 1) * 512)
            ps = psum.tile([128, 512], F32, name="ps_acc", tag="acc", bufs=2)
            for m in range(PAIRS):
                nc.tensor.matmul(
                    ps[:],
                    oT_t[m][:, tsl],
                    wo_t[m][:, nsl],
                    start=(m == 0),
                    stop=(m == PAIRS - 1),
                )
            # pair the two 512-col halves into one [128,1024] staging tile
            # so the output DMA moves 2KB lines (f16 512-col would be 1KB)
            if ncx == 0:
                ot_full[tb] = opool.tile([128, 1024], F16, name="ot",
                                         tag="ot", bufs=3)
            ot = ot_full[tb]
            # trailing chains run after the last exp: split the staging
            # copies between ScalarE and VectorE so they drain in parallel
            # (16 serial DVE copies were the tail's critical path). The
            # in-window chains must NOT touch the scalar queue -- its exps.
            if trailing and ncx == 0:
                nc.scalar.copy(ot[:, nsl], ps[:])
            else:
                nc.vector.tensor_copy(ot[:, nsl], ps[:])
            if ncx == 1:
                # two half-DMAs on different queues: a single 256KB piece
                # rides one DMA ring (~12us) and becomes pure tail latency
                for hf in (0, 1):
                    if trailing:
                        q = oq_cycle[(tb * 2 + hf) % 3]
                    else:
                        q = (nc.sync, nc.gpsimd)[(tb * 2 + hf) % 2]
                    q.dma_start(out[tsl, hf * 512:(hf + 1) * 512],
                                ot[:, hf * 512:(hf + 1) * 512])

        # upfront: k0ic0 (earliest-landing inputs), then the v chains (fill
        # the head's DMA-paced Tensor idle), then q0ic0 (latest inputs)
        proj_chain(0, 1, 0)   # k0 ic0
        for tb in range(TB):
            v_chain(tb)
        proj_chain(0, 0, 0)   # q0 ic0

        # ---------------- attention ----------------
        # fill[(p, ih)]: chain thunks run at jb==2 and jb==5 -- the chain a
        # scores MM depends on is always EMITTED earlier, and the Tensor
        # queue executes in order, so this can never deadlock.
        oT_t = [apool.tile([128, T], BF16, name=f"oT{m}", tag=f"oT{m}")
                for m in range(PAIRS)]
        oq_cycle[:] = [nc.scalar, nc.sync, nc.gpsimd]
        # fill[(p, ih)]: {jb: [thunk specs]} run after that jb's scores/exp.
        # The chain a later scores/AV MM depends on is always EMITTED
        # earlier, and the Tensor queue executes in order -> no deadlock.
        fill = {
            (0, 0): {2: [(0, 1, 1)], 5: [(0, 0, 1)]},
            (0, 1): {2: [(1, 0, 0)], 5: [(1, 1, 0)]},
            (1, 0): {2: [(1, 1, 1)], 5: [(1, 0, 1)]},
            (1, 1): {2: [(2, 0, 0)], 5: [(2, 1, 0)]},
            (2, 0): {2: [(2, 1, 1)], 5: [(2, 0, 1)]},
            (2, 1): {2: [(3, 0, 0)], 5: [(3, 1, 0)]},
            (3, 0): {2: [(3, 1, 1)], 5: [(3, 0, 1)]},
            # out-proj tb 0-3 read only the ih0 token half of oT, so they
            # interleave into the last attention block; their DMAs drain
            # during it
            # start at jb2 (the (3,0) norm must finish first) and keep to
            # ONE chain per jb -- two back-to-back chains starve the exps
            (3, 1): {2: [("out", 0, 0)], 3: [("out", 0, 1)],
                     4: [("out", 1, 0)], 5: [("out", 1, 1)],
                     6: [("out", 2, 0)], 7: [("out", 2, 1)]},
        }
        # Blocks run back-to-back with one-step lookahead: the next
        # block's scores(jb0)+exp(jb0) are emitted BEFORE this block's
        # AV-drain + norm, so neither engine idles at block boundaries.
        prev_tail = [None]

        def block(p, ih):
            oT = oT_t[p]
            isl = slice(ih * 512, (ih + 1) * 512)
            po_box = [None]
            pend = []
            slots = fill[(p, ih)]
            for jb in range(TB):
                jsl = slice(jb * 128, (jb + 1) * 128)
                # one [128, 1024] psum tile: bank0 = head 2p, bank1 = 2p+1
                pss = psum.tile([128, 1024], F32, name="ps_s", tag="sc",
                                bufs=2)
                for h in (0, 1):
                    hsl = slice(h * 64, (h + 1) * 64)
                    nc.tensor.matmul(
                        pss[:, h * 512:(h + 1) * 512],
                        kT_t[p][hsl, jsl],
                        qT_t[p][hsl, isl],
                    )
                # one exp for both heads; mask bias is per-partition (= j)
                et = epool.tile([128, 1024], BF16, name="et", tag="et",
                                bufs=6)
                nc.scalar.activation(et[:], pss[:], Exp,
                                     bias=mask_t[:, jb:jb + 1],
                                     scale=SCALE)
                if jb == 0 and prev_tail[0] is not None:
                    prev_tail[0]()
                    prev_tail[0] = None
                pend.append((jb, et))
                if len(pend) > AV_LAG:
                    if po_box[0] is None:
                        po_box[0] = psum.tile([128, 1024], F32, name="po",
                                              tag="po", bufs=1)
                    _av(nc, pend.pop(0), p, po_box[0], v_t)
                for s in slots.get(jb, ()):
                    if s[0] == "v":
                        v_chain(s[1])
                    elif s[0] == "out":
                        out_chain(s[1], s[2], oT_t)
                    else:
                        proj_chain(*s)

            def tail():
                po = po_box[0]
                while pend:
                    _av(nc, pend.pop(0), p, po, v_t)
                # normalize. po bank h = [s_h (64 rows) | o_h (64 rows)]
                # rb = 1/s on VectorE (~51 ULP approx; sums are O(1..1e3));
                # recip inputs MUST be partition-offset-0 APs
                rb_e = npool.tile([CH, 512], F32, name="rb_e", tag="rb",
                                  bufs=4)
                nc.vector.reciprocal_approx_fast(rb_e[:], po[0:CH, 0:512])
                nc.vector.tensor_mul(oT[0:CH, isl], po[CH:128, 0:512],
                                     rb_e[:])
                rb_o = npool.tile([CH, 512], F32, name="rb_o", tag="rb",
                                  bufs=4)
                nc.vector.reciprocal_approx_fast(rb_o[:], po[0:CH, 512:1024])
                nc.vector.tensor_mul(oT[CH:128, isl], po[CH:128, 512:1024],
                                     rb_o[:])

            prev_tail[0] = tail

        for p in range(PAIRS):
            for ih in range(2):
                block(p, ih)
        prev_tail[0]()

        # ---------------- out = oT.T @ Wo (rest) ----------------
        # tb3 + tb 4..7; tb4+ need the p3-ih1 norm and run while VectorE
        # finishes it
        for tb in range(3, TB):
            for ncx in range(2):
                out_chain(tb, ncx, oT_t, trailing=True)


def _av(nc, item, p, po, v_t):
    jb, et = item
    for h in (0, 1):
        # head 2p+h stationary: [ones|v_h] -> out rows [sums | o_h]
        csl = slice(p * VW + h * 128, p * VW + h * 128 + 128)
        nc.tensor.matmul(
            po[:, h * 512:(h + 1) * 512],
            v_t[jb][:, csl],
            et[:, h * 512:(h + 1) * 512],
            start=(jb == 0),
            stop=(jb == TB - 1),
        )


def build_nc():
    nc = bacc.Bacc("TRN2", target_bir_lowering=False, debug=False,
                   num_devices=N_CORES)
    with tile.TileContext(nc) as tc:
        _emit(tc)
    nc.compile()
    return nc


def _get_compiled():
    if "nc" not in _CACHE:
        _CACHE["nc"] = build_nc()
    return _CACHE["nc"]


def make_in_maps(x_q, x_kv, pad_mask):
    ones = np.ones((128, PAIRS * 2 * CH), BF_NP)
    in_maps = []
    for c in range(N_CORES):
        b, g = divmod(c, 2)
        gs = slice(g * G, (g + 1) * G)
        mbias = np.where(pad_mask[b], np.float32(NEG), np.float32(0.0))
        in_maps.append({
            "xqT": np.ascontiguousarray(x_q[b].T).astype(BF_NP),
            "xkvT": np.ascontiguousarray(x_kv[b].T).astype(BF_NP),
            "wq": _W["q"][:, gs].astype(BF_NP),
            "wk": _W["k"][:, gs].astype(BF_NP),
            "wv": _W["v"][:, gs].astype(BF_NP),
            "wo": np.ascontiguousarray(_W["o"][gs, :]).astype(BF_NP),
            "mb": np.ascontiguousarray(mbias.astype(np.float32).reshape(TB, 128).T),
            "vones": ones,
        })
    return in_maps


_W = {}


def kernel(x_q, x_kv, pad_mask, Wq, Wk, Wv, Wo, bo):
    global LAST_RESULTS
    x_q = np.asarray(x_q, dtype=np.float32)
    x_kv = np.asarray(x_kv, dtype=np.float32)
    pad_mask = np.asarray(pad_mask)
    _W["q"] = np.asarray(Wq, dtype=np.float32)
    _W["k"] = np.asarray(Wk, dtype=np.float32)
    _W["v"] = np.asarray(Wv, dtype=np.float32)
    _W["o"] = np.asarray(Wo, dtype=np.float32)
    bo = np.asarray(bo, dtype=np.float32)

    nc = _get_compiled()
    in_maps = make_in_maps(x_q, x_kv, pad_mask)
    res = bass_utils.run_bass_kernel_spmd(nc, in_maps, list(range(N_CORES)))
    LAST_RESULTS = res
    outp = np.zeros((B, T, D), np.float32)
    for b in range(B):
        outp[b] = (res.results[2 * b]["out"].astype(np.float32)
                   + res.results[2 * b + 1]["out"].astype(np.float32))
    outp += bo[None, None, :]
    return outp


# revision 31
# speedup vs baseline: 1.0095x; 1.0095x over previous
"""Multi-head attention, tensor-parallel over heads x data-parallel over batch.

8 NeuronCores: core c handles batch b=c//2, head-group g=c%2 (8 heads, 512 chans).
Each core computes its head-group's attention + partial output projection;
the two partials per batch are summed on the host (row-parallel Wo unshard).

Design (vs the fp32r baseline: 234us -> ~163us):
  - every matmul operand is bf16 (halves DMA + LDWEIGHTS; PSUM stays fp32;
    rel err 7e-3 vs the 2e-2 gate)
  - softmax 1/sum on VectorE via reciprocal_approx_fast, so ScalarE runs
    ONLY Exp (one ACT table set, no FIFO clog from the old ln/exp norm).
    recip silently misreads partition-offset APs, hence the [ones|v]
    stationary packing that puts the sums on partitions 0:64.
  - fp16 output with paired [128,1024] staging (2KB DMA lines), halves the
    output-drain tail
  - only k0/q0 ic0 projection chains run upfront; every other chain (v,
    qk of later pairs, out-proj of the ih0 token half) is interleaved into
    the attention jb-loops via the fill schedule, so TensorE streams
    gap-free behind the ~40us DMA-bandwidth-bound input window
  - one-step block lookahead: the next block's scores+exp are emitted
    before this block's AV-drain + norm (no engine idles at boundaries)
  - input DMA in ~128KB pieces over the sync+gpsimd queues only (each
    dma_start rides ONE ~22GB/s ring; scalar-queue issues would gate the
    first exp); three waves ordered by first use

Per-core dataflow:
  qT/kT = W.T @ xT        [chan, tok] transposed projections
  v     = xkv @ Wv        [tok, chan] natural projection, stored per head
                          pair as [ones|v_even|ones|v_odd] (64 cols each)
  scoresT[j,i] = k.q      row-group packed pairs -> one [128,1024] psum
  expT  = exp(s*scale + maskbias)   one ScalarE op per (pair, ih, jb), bf16 out
  AV:   lhsT = [ones|v_h] (M=128) -> po rows = [sums replicated | o_h]
  norm: rb = recip(sums) on VectorE, oT = po * rb on VectorE (bf16 out)
  out   = oT.T @ Wo       partial output projection
"""

import numpy as np

import concourse.bacc as bacc
import concourse.mybir as mybir
import concourse.tile as tile
from concourse import bass_utils

B = 4
T = 1024          # tokens (N = L)
D = 1024          # model dim
CH = 64           # channels per head
G = 512           # channels per head-group (8 heads)
SCALE = CH ** -0.5
NEG = -30000.0    # mask bias (exp(x + NEG) == 0)
F32 = mybir.dt.float32
F16 = mybir.dt.float16
BF16 = mybir.dt.bfloat16
BF_NP = mybir.dt.np(mybir.dt.bfloat16)
F16_NP = mybir.dt.np(mybir.dt.float16)

N_CORES = 8
KB = 8            # 128-row contraction blocks over D
TB = 8            # 128-token blocks
PAIRS = 4         # head pairs per core
VW = 256          # v-tile columns per head pair: [ones|v_even|ones|v_odd]
AV_LAG = 2        # software-pipeline depth: AV trails scores/exp by this many jb

LAST_RESULTS = None
_CACHE = {}


def _emit(tc):
    nc = tc.nc
    xqT = nc.dram_tensor("xqT", [D, T], BF16, kind="ExternalInput").ap()
    xkvT = nc.dram_tensor("xkvT", [D, T], BF16, kind="ExternalInput").ap()
    wq = nc.dram_tensor("wq", [D, G], BF16, kind="ExternalInput").ap()
    wk = nc.dram_tensor("wk", [D, G], BF16, kind="ExternalInput").ap()
    wv = nc.dram_tensor("wv", [D, G], BF16, kind="ExternalInput").ap()
    wo = nc.dram_tensor("wo", [G, D], BF16, kind="ExternalInput").ap()
    mb = nc.dram_tensor("mb", [128, TB], F32, kind="ExternalInput").ap()
    vones = nc.dram_tensor("vones", [128, PAIRS * 2 * CH], BF16,
                           kind="ExternalInput").ap()
    out = nc.dram_tensor("out", [T, D], F16, kind="ExternalOutput").ap()

    Exp = mybir.ActivationFunctionType.Exp

    with (
        tc.tile_pool(name="wpool", bufs=1) as wpool,
        tc.tile_pool(name="xpool", bufs=1) as xpool,
        tc.tile_pool(name="apool", bufs=1) as apool,
        tc.tile_pool(name="epool", bufs=1) as epool,
        tc.tile_pool(name="npool", bufs=1) as npool,
        tc.tile_pool(name="opool", bufs=4) as opool,
        tc.tile_pool(name="psum", bufs=1, space="PSUM") as psum,
    ):
        # ---------------- input DMA ----------------
        # One dma_start rides ONE queue (~23 GB/s), so transfers are split
        # per 128KB piece and spread over the sync/scalar/gpsimd queues in
        # first-use order. Issue cost is ~0.6-0.9us per dma_start, so each
        # queue carries only what it must before its compute begins.
        mask_t = wpool.tile([128, TB], F32, name="mask_t", tag="mask")

        xkv_b = xpool.tile([128, KB * T], BF16, name="xkv", tag="xkv")
        xq_b = xpool.tile([128, KB * T], BF16, name="xq", tag="xq")
        xkv_src = xkvT.rearrange("(kb p) t -> p kb t", p=128)
        xq_src = xqT.rearrange("(kb p) t -> p kb t", p=128)
        xkv_dst = xkv_b.rearrange("p (kb t) -> p kb t", t=T)
        xq_dst = xq_b.rearrange("p (kb t) -> p kb t", t=T)
        xkv_t = [xkv_b[:, k * T:(k + 1) * T] for k in range(KB)]
        xq_t = [xq_b[:, k * T:(k + 1) * T] for k in range(KB)]

        wv_b = wpool.tile([128, KB * G], BF16, name="wv", tag="wv")
        wq_b = wpool.tile([128, KB * G], BF16, name="wq", tag="wq")
        wk_b = wpool.tile([128, KB * G], BF16, name="wk", tag="wk")
        wv_t = [wv_b[:, k * G:(k + 1) * G] for k in range(KB)]
        wq_t = [wq_b[:, k * G:(k + 1) * G] for k in range(KB)]
        wk_t = [wk_b[:, k * G:(k + 1) * G] for k in range(KB)]
        wo_b = wpool.tile([128, PAIRS * D], BF16, name="wo", tag="wo")
        wo_t = [wo_b[:, m * D:(m + 1) * D] for m in range(PAIRS)]

        v_t = [apool.tile([128, PAIRS * VW], BF16, name=f"v{tb}",
                          tag=f"v{tb}") for tb in range(TB)]
        vo_src = vones.rearrange("p (pb f c) -> p pb f c", f=2, c=CH)
        wdst = {id(wv_b): wv_b.rearrange("p (kb g) -> p kb g", g=G),
                id(wq_b): wq_b.rearrange("p (kb g) -> p kb g", g=G),
                id(wk_b): wk_b.rearrange("p (kb g) -> p kb g", g=G)}
        wsrc = {id(wv_b): wv.rearrange("(kb p) g -> p kb g", p=128),
                id(wq_b): wq.rearrange("(kb p) g -> p kb g", p=128),
                id(wk_b): wk.rearrange("(kb p) g -> p kb g", p=128)}
        wo_dst = wo_b.rearrange("p (m d) -> p m d", d=D)
        wo_src2 = wo.rearrange("(m p) d -> p m d", p=128)

        # Waves balanced across the sync/gpsimd queues ONLY -- a DMA issue
        # on the scalar queue would gate the first exp behind the issue
        # stream. Many ~128KB pieces: each dma_start rides one DMA ring, so
        # small pieces across many rings is what parallelizes the head.
        # wave 1: what k0ic0/q0ic0 + scores(p0,ih0,jb0-3) need
        nc.sync.dma_start(mask_t[:], mb[:])
        qq = (nc.sync, nc.gpsimd)
        # k0ic0's very first matmul needs only wk[0][:,0:128] and the token
        # half of xkv[0]: tiny first pieces land in ~5-9us instead of the
        # 12-17us a full 256KB single-ring piece takes, starting the PE (and
        # its HAM warmup) that much earlier
        nc.gpsimd.dma_start(wdst[id(wk_b)][:, 0, 0:128],
                            wsrc[id(wk_b)][:, 0, 0:128])
        nc.sync.dma_start(xkv_dst[:, 0, 0:512], xkv_src[:, 0, 0:512])
        nc.gpsimd.dma_start(wdst[id(wk_b)][:, 0, 128:512],
                            wsrc[id(wk_b)][:, 0, 128:512])
        nc.sync.dma_start(xkv_dst[:, 0, 512:1024], xkv_src[:, 0, 512:1024])
        # full-row x chunks: 2KB lines run at 21.9 GB/s/ring vs 16.4 for 1KB
        for k in range(1, KB):
            qq[k % 2].dma_start(xkv_dst[:, k, :], xkv_src[:, k, :])
        for k in range(1, KB):
            qq[(k + 1) % 2].dma_start(wdst[id(wk_b)][:, k, :],
                                      wsrc[id(wk_b)][:, k, :])
        # wv in wave 1: the v chains run upfront, in the head's otherwise
        # DMA-paced Tensor idle, which empties ~15us of matmuls out of the
        # (Tensor-bound) attention window
        for k in range(KB):
            qq[k % 2].dma_start(wdst[id(wv_b)][:, k, :], wsrc[id(wv_b)][:, k, :])
        for k in range(KB):
            qq[(k + 1) % 2].dma_start(xq_dst[:, k, :], xq_src[:, k, :])
        for k in range(KB):
            qq[k % 2].dma_start(wdst[id(wq_b)][:, k, :],
                                wsrc[id(wq_b)][:, k, :])
        # wave 2
        for m in range(PAIRS):
            qq[m % 2].dma_start(wo_dst[:, m, :], wo_src2[:, m, :])
        # ones blocks via GpSimd memset -- the DMA pattern moved 0.5MB in
        # 128B packets (7 GB/s) through the head's critical DMA window
        for tb in range(TB):
            v2c = v_t[tb].rearrange("p (pb f c) -> p pb f c", f=2, c=128)
            nc.gpsimd.memset(v2c[:, :, :, 0:CH], 1.0)

        # ---------------- v = xkv @ Wv (natural layout) ----------------
        def v_chain(tb):
            ps = psum.tile([128, 512], F32, name="ps_acc", tag="acc", bufs=2)
            for k in range(KB):
                nc.tensor.matmul(
                    ps[:],
                    xkv_t[k][:, tb * 128:(tb + 1) * 128],
                    wv_t[k][:],
                    start=(k == 0),
                    stop=(k == KB - 1),
                )
            v4 = v_t[tb].rearrange("p (pb four c) -> p pb four c",
                                   four=4, c=CH)
            ps3 = ps.rearrange("p (pb two c) -> p pb two c", two=2, c=CH)
            nc.vector.tensor_copy(v4[:, :, 1:2, :], ps3[:, :, 0:1, :])
            nc.vector.tensor_copy(v4[:, :, 3:4, :], ps3[:, :, 1:2, :])

        # (v chains are emitted between k0ic0 and q0ic0 below)

        # ---------------- qT/kT projection chains ----------------
        qT_t = [apool.tile([128, T], BF16, name=f"qT{m}", tag=f"qT{m}")
                for m in range(PAIRS)]
        kT_t = [apool.tile([128, T], BF16, name=f"kT{m}", tag=f"kT{m}")
                for m in range(PAIRS)]

        def proj_chain(p, which, ic):
            src, w_t, dst = ((xq_t, wq_t, qT_t), (xkv_t, wk_t, kT_t))[which]
            csl = slice(ic * 512, (ic + 1) * 512)
            ps = psum.tile([128, 512], F32, name="ps_acc", tag="acc", bufs=2)
            for k in range(KB):
                nc.tensor.matmul(
                    ps[:],
                    w_t[k][:, p * 128:(p + 1) * 128],
                    src[k][:, csl],
                    start=(k == 0),
                    stop=(k == KB - 1),
                )
            nc.vector.tensor_copy(dst[p][:, csl], ps[:])

        # out-projection chain; tb 0..3 read only the ih=0 half of oT, so
        # they can interleave into pair 3's attention
        oq_cycle = [None, None, None]  # filled below (queues)

        ot_full = {}

        def out_chain(tb, ncx, oT_t, trailing=False):
            tsl = slice(tb * 128, (tb + 1) * 128)
            nsl = slice(ncx * 512, (ncx + 1) * 512)
            ps = psum.tile([128, 512], F32, name="ps_acc", tag="acc", bufs=2)
            for m in range(PAIRS):
                nc.tensor.matmul(
                    ps[:],
                    oT_t[m][:, tsl],
                    wo_t[m][:, nsl],
                    start=(m == 0),
                    stop=(m == PAIRS - 1),
                )
            # pair the two 512-col halves into one [128,1024] staging tile
            # so the output DMA moves 2KB lines (f16 512-col would be 1KB)
            if ncx == 0:
                ot_full[tb] = opool.tile([128, 1024], F16, name="ot",
                                         tag="ot", bufs=3)
            ot = ot_full[tb]
            # trailing chains run after the last exp: split the staging
            # copies between ScalarE and VectorE so they drain in parallel
            # (16 serial DVE copies were the tail's critical path). The
            # in-window chains must NOT touch the scalar queue -- its exps.
            if trailing and ncx == 0:
                nc.scalar.copy(ot[:, nsl], ps[:])
            else:
                nc.vector.tensor_copy(ot[:, nsl], ps[:])
            if ncx == 1:
                # two half-DMAs on different queues: a single 256KB piece
                # rides one DMA ring (~12us) and becomes pure tail latency
                for hf in (0, 1):
                    if trailing:
                        q = oq_cycle[(tb * 2 + hf) % 3]
                    else:
                        q = (nc.sync, nc.gpsimd)[(tb * 2 + hf) % 2]
                    q.dma_start(out[tsl, hf * 512:(hf + 1) * 512],
                                ot[:, hf * 512:(hf + 1) * 512])

        # upfront: k0ic0 (earliest-landing inputs), then the v chains (fill
        # the head's DMA-paced Tensor idle), then q0ic0 (latest inputs)
        proj_chain(0, 1, 0)   # k0 ic0
        for tb in range(TB):
            v_chain(tb)
        proj_chain(0, 0, 0)   # q0 ic0

        # ---------------- attention ----------------
        # fill[(p, ih)]: chain thunks run at jb==2 and jb==5 -- the chain a
        # scores MM depends on is always EMITTED earlier, and the Tensor
        # queue executes in order, so this can never deadlock.
        oT_t = [apool.tile([128, T], BF16, name=f"oT{m}", tag=f"oT{m}")
                for m in range(PAIRS)]
        oq_cycle[:] = [nc.scalar, nc.sync, nc.gpsimd]
        # fill[(p, ih)]: {jb: [thunk specs]} run after that jb's scores/exp.
        # The chain a later scores/AV MM depends on is always EMITTED
        # earlier, and the Tensor queue executes in order -> no deadlock.
        fill = {
            (0, 0): {2: [(0, 1, 1)], 5: [(0, 0, 1)]},
            (0, 1): {2: [(1, 0, 0)], 5: [(1, 1, 0)]},
            (1, 0): {2: [(1, 1, 1)], 5: [(1, 0, 1)]},
            (1, 1): {2: [(2, 0, 0)], 5: [(2, 1, 0)]},
            (2, 0): {2: [(2, 1, 1)], 5: [(2, 0, 1)]},
            (2, 1): {2: [(3, 0, 0)], 5: [(3, 1, 0)]},
            (3, 0): {2: [(3, 1, 1)], 5: [(3, 0, 1)]},
            # out-proj tb 0-3 read only the ih0 token half of oT, so they
            # interleave into the last attention block; their DMAs drain
            # during it
            # start at jb2: the (3,0) norm (DVE, ~2.8us) must finish
            # before the first chain's m=3 matmul, or it stalls the queue
            (3, 1): {2: [("out", 0, 0)], 3: [("out", 0, 1)],
                     4: [("out", 1, 0), ("out", 1, 1)],
                     5: [("out", 2, 0), ("out", 2, 1)],
                     6: [("out", 3, 0)], 7: [("out", 3, 1)]},
        }
        # Blocks run back-to-back with one-step lookahead: the next
        # block's scores(jb0)+exp(jb0) are emitted BEFORE this block's
        # AV-drain + norm, so neither engine idles at block boundaries.
        prev_tail = [None]

        def block(p, ih):
            oT = oT_t[p]
            isl = slice(ih * 512, (ih + 1) * 512)
            po_box = [None]
            pend = []
            slots = fill[(p, ih)]
            for jb in range(TB):
                jsl = slice(jb * 128, (jb + 1) * 128)
                # one [128, 1024] psum tile: bank0 = head 2p, bank1 = 2p+1
                pss = psum.tile([128, 1024], F32, name="ps_s", tag="sc",
                                bufs=2)
                for h in (0, 1):
                    hsl = slice(h * 64, (h + 1) * 64)
                    nc.tensor.matmul(
                        pss[:, h * 512:(h + 1) * 512],
                        kT_t[p][hsl, jsl],
                        qT_t[p][hsl, isl],
                    )
                # one exp for both heads; mask bias is per-partition (= j)
                et = epool.tile([128, 1024], BF16, name="et", tag="et",
                                bufs=6)
                nc.scalar.activation(et[:], pss[:], Exp,
                                     bias=mask_t[:, jb:jb + 1],
                                     scale=SCALE)
                if jb == 0 and prev_tail[0] is not None:
                    prev_tail[0]()
                    prev_tail[0] = None
                pend.append((jb, et))
                if len(pend) > AV_LAG:
                    if po_box[0] is None:
                        po_box[0] = psum.tile([128, 1024], F32, name="po",
                                              tag="po", bufs=1)
                    _av(nc, pend.pop(0), p, po_box[0], v_t)
                for s in slots.get(jb, ()):
                    if s[0] == "v":
                        v_chain(s[1])
                    elif s[0] == "out":
                        out_chain(s[1], s[2], oT_t)
                    else:
                        proj_chain(*s)

            def tail():
                po = po_box[0]
                while pend:
                    _av(nc, pend.pop(0), p, po, v_t)
                # normalize. po bank h = [s_h (64 rows) | o_h (64 rows)]
                # rb = 1/s on VectorE (~51 ULP approx; sums are O(1..1e3));
                # recip inputs MUST be partition-offset-0 APs
                rb_e = npool.tile([CH, 512], F32, name="rb_e", tag="rb",
                                  bufs=4)
                nc.vector.reciprocal_approx_fast(rb_e[:], po[0:CH, 0:512])
                nc.vector.tensor_mul(oT[0:CH, isl], po[CH:128, 0:512],
                                     rb_e[:])
                rb_o = npool.tile([CH, 512], F32, name="rb_o", tag="rb",
                                  bufs=4)
                nc.vector.reciprocal_approx_fast(rb_o[:], po[0:CH, 512:1024])
                nc.vector.tensor_mul(oT[CH:128, isl], po[CH:128, 512:1024],
                                     rb_o[:])

            prev_tail[0] = tail

        for p in range(PAIRS):
            for ih in range(2):
                block(p, ih)
        prev_tail[0]()

        # ---------------- out = oT.T @ Wo (rest) ----------------
        # tb 4..7 need the p3-ih1 norm and run while VectorE finishes it
        for tb in range(4, TB):
            for ncx in range(2):
                out_chain(tb, ncx, oT_t, trailing=True)


def _av(nc, item, p, po, v_t):
    jb, et = item
    for h in (0, 1):
        # head 2p+h stationary: [ones|v_h] -> out rows [sums | o_h]
        csl = slice(p * VW + h * 128, p * VW + h * 128 + 128)
        nc.tensor.matmul(
            po[:, h * 512:(h + 1) * 512],
            v_t[jb][:, csl],
            et[:, h * 512:(h + 1) * 512],
            start=(jb == 0),
            stop=(jb == TB - 1),
        )


def build_nc():
    nc = bacc.Bacc("TRN2", target_bir_lowering=False, debug=False,
                   num_devices=N_CORES)
    with tile.TileContext(nc) as tc:
        _emit(tc)
    nc.compile()
    return nc


def _get_compiled():
    if "nc" not in _CACHE:
        _CACHE["nc"] = build_nc()
    return _CACHE["nc"]


def make_in_maps(x_q, x_kv, pad_mask):
    ones = np.ones((128, PAIRS * 2 * CH), BF_NP)
    in_maps = []
    for c in range(N_CORES):
        b, g = divmod(c, 2)
        gs = slice(g * G, (g + 1) * G)
        mbias = np.where(pad_mask[b], np.float32(NEG), np.float32(0.0))
        in_maps.append({
            "xqT": np.ascontiguousarray(x_q[b].T).astype(BF_NP),
            "xkvT": np.ascontiguousarray(x_kv[b].T).astype(BF_NP),
            "wq": _W["q"][:, gs].astype(BF_NP),
            "wk": _W["k"][:, gs].astype(BF_NP),
            "wv": _W["v"][:, gs].astype(BF_NP),
            "wo": np.ascontiguousarray(_W["o"][gs, :]).astype(BF_NP),
            "mb": np.ascontiguousarray(mbias.astype(np.float32).reshape(TB, 128).T),
            "vones": ones,
        })
    return in_maps


_W = {}


def kernel(x_q, x_kv, pad_mask, Wq, Wk, Wv, Wo, bo):
    global LAST_RESULTS
    x_q = np.asarray(x_q, dtype=np.float32)
    x_kv = np.asarray(x_kv, dtype=np.float32)
    pad_mask = np.asarray(pad_mask)
    _W["q"] = np.asarray(Wq, dtype=np.float32)
    _W["k"] = np.asarray(Wk, dtype=np.float32)
    _W["v"] = np.asarray(Wv, dtype=np.float32)
    _W["o"] = np.asarray(Wo, dtype=np.float32)
    bo = np.asarray(bo, dtype=np.float32)

    nc = _get_compiled()
    in_maps = make_in_maps(x_q, x_kv, pad_mask)
    res = bass_utils.run_bass_kernel_spmd(nc, in_maps, list(range(N_CORES)))
    LAST_RESULTS = res
    outp = np.zeros((B, T, D), np.float32)
    for b in range(B):
        outp[b] = (res.results[2 * b]["out"].astype(np.float32)
                   + res.results[2 * b + 1]["out"].astype(np.float32))
    outp += bo[None, None, :]
    return outp


# revision 32
# speedup vs baseline: 1.0247x; 1.0150x over previous
"""Multi-head attention, tensor-parallel over heads x data-parallel over batch.

8 NeuronCores: core c handles batch b=c//2, head-group g=c%2 (8 heads, 512 chans).
Each core computes its head-group's attention + partial output projection;
the two partials per batch are summed on the host (row-parallel Wo unshard).

Design (vs the fp32r baseline: 234us -> ~163us):
  - every matmul operand is bf16 (halves DMA + LDWEIGHTS; PSUM stays fp32;
    rel err 7e-3 vs the 2e-2 gate)
  - softmax 1/sum on VectorE via reciprocal_approx_fast, so ScalarE runs
    ONLY Exp (one ACT table set, no FIFO clog from the old ln/exp norm).
    recip silently misreads partition-offset APs, hence the [ones|v]
    stationary packing that puts the sums on partitions 0:64.
  - fp16 output with paired [128,1024] staging (2KB DMA lines), halves the
    output-drain tail
  - only k0/q0 ic0 projection chains run upfront; every other chain (v,
    qk of later pairs, out-proj of the ih0 token half) is interleaved into
    the attention jb-loops via the fill schedule, so TensorE streams
    gap-free behind the ~40us DMA-bandwidth-bound input window
  - one-step block lookahead: the next block's scores+exp are emitted
    before this block's AV-drain + norm (no engine idles at boundaries)
  - input DMA in ~128KB pieces over the sync+gpsimd queues only (each
    dma_start rides ONE ~22GB/s ring; scalar-queue issues would gate the
    first exp); three waves ordered by first use

Per-core dataflow:
  qT/kT = W.T @ xT        [chan, tok] transposed projections
  v     = xkv @ Wv        [tok, chan] natural projection, stored per head
                          pair as [ones|v_even|ones|v_odd] (64 cols each)
  scoresT[j,i] = k.q      row-group packed pairs -> one [128,1024] psum
  expT  = exp(s*scale + maskbias)   one ScalarE op per (pair, ih, jb), bf16 out
  AV:   lhsT = [ones|v_h] (M=128) -> po rows = [sums replicated | o_h]
  norm: rb = recip(sums) on VectorE, oT = po * rb on VectorE (bf16 out)
  out   = oT.T @ Wo       partial output projection
"""

import numpy as np

import concourse.bacc as bacc
import concourse.mybir as mybir
import concourse.tile as tile
from concourse import bass_utils

B = 4
T = 1024          # tokens (N = L)
D = 1024          # model dim
CH = 64           # channels per head
G = 512           # channels per head-group (8 heads)
SCALE = CH ** -0.5
NEG = -30000.0    # mask bias (exp(x + NEG) == 0)
F32 = mybir.dt.float32
F16 = mybir.dt.float16
BF16 = mybir.dt.bfloat16
BF_NP = mybir.dt.np(mybir.dt.bfloat16)
F16_NP = mybir.dt.np(mybir.dt.float16)

N_CORES = 8
KB = 8            # 128-row contraction blocks over D
TB = 8            # 128-token blocks
PAIRS = 4         # head pairs per core
VW = 256          # v-tile columns per head pair: [ones|v_even|ones|v_odd]
AV_LAG = 2        # software-pipeline depth: AV trails scores/exp by this many jb

LAST_RESULTS = None
_CACHE = {}


def _emit(tc):
    nc = tc.nc
    xqT = nc.dram_tensor("xqT", [D, T], BF16, kind="ExternalInput").ap()
    xkvT = nc.dram_tensor("xkvT", [D, T], BF16, kind="ExternalInput").ap()
    wq = nc.dram_tensor("wq", [D, G], BF16, kind="ExternalInput").ap()
    wk = nc.dram_tensor("wk", [D, G], BF16, kind="ExternalInput").ap()
    wv = nc.dram_tensor("wv", [D, G], BF16, kind="ExternalInput").ap()
    wo = nc.dram_tensor("wo", [G, D], BF16, kind="ExternalInput").ap()
    mb = nc.dram_tensor("mb", [128, TB], F32, kind="ExternalInput").ap()
    vones = nc.dram_tensor("vones", [128, PAIRS * 2 * CH], BF16,
                           kind="ExternalInput").ap()
    out = nc.dram_tensor("out", [T, D], F16, kind="ExternalOutput").ap()

    Exp = mybir.ActivationFunctionType.Exp

    with (
        tc.tile_pool(name="wpool", bufs=1) as wpool,
        tc.tile_pool(name="xpool", bufs=1) as xpool,
        tc.tile_pool(name="apool", bufs=1) as apool,
        tc.tile_pool(name="epool", bufs=1) as epool,
        tc.tile_pool(name="npool", bufs=1) as npool,
        tc.tile_pool(name="opool", bufs=4) as opool,
        tc.tile_pool(name="psum", bufs=1, space="PSUM") as psum,
    ):
        # ---------------- input DMA ----------------
        # One dma_start rides ONE queue (~23 GB/s), so transfers are split
        # per 128KB piece and spread over the sync/scalar/gpsimd queues in
        # first-use order. Issue cost is ~0.6-0.9us per dma_start, so each
        # queue carries only what it must before its compute begins.
        mask_t = wpool.tile([128, TB], F32, name="mask_t", tag="mask")

        xkv_b = xpool.tile([128, KB * T], BF16, name="xkv", tag="xkv")
        xq_b = xpool.tile([128, KB * T], BF16, name="xq", tag="xq")
        xkv_src = xkvT.rearrange("(kb p) t -> p kb t", p=128)
        xq_src = xqT.rearrange("(kb p) t -> p kb t", p=128)
        xkv_dst = xkv_b.rearrange("p (kb t) -> p kb t", t=T)
        xq_dst = xq_b.rearrange("p (kb t) -> p kb t", t=T)
        xkv_t = [xkv_b[:, k * T:(k + 1) * T] for k in range(KB)]
        xq_t = [xq_b[:, k * T:(k + 1) * T] for k in range(KB)]

        wv_b = wpool.tile([128, KB * G], BF16, name="wv", tag="wv")
        wq_b = wpool.tile([128, KB * G], BF16, name="wq", tag="wq")
        wk_b = wpool.tile([128, KB * G], BF16, name="wk", tag="wk")
        wv_t = [wv_b[:, k * G:(k + 1) * G] for k in range(KB)]
        wq_t = [wq_b[:, k * G:(k + 1) * G] for k in range(KB)]
        wk_t = [wk_b[:, k * G:(k + 1) * G] for k in range(KB)]
        wo_b = wpool.tile([128, PAIRS * D], BF16, name="wo", tag="wo")
        wo_t = [wo_b[:, m * D:(m + 1) * D] for m in range(PAIRS)]

        v_t = [apool.tile([128, PAIRS * VW], BF16, name=f"v{tb}",
                          tag=f"v{tb}") for tb in range(TB)]
        vo_src = vones.rearrange("p (pb f c) -> p pb f c", f=2, c=CH)
        wdst = {id(wv_b): wv_b.rearrange("p (kb g) -> p kb g", g=G),
                id(wq_b): wq_b.rearrange("p (kb g) -> p kb g", g=G),
                id(wk_b): wk_b.rearrange("p (kb g) -> p kb g", g=G)}
        wsrc = {id(wv_b): wv.rearrange("(kb p) g -> p kb g", p=128),
                id(wq_b): wq.rearrange("(kb p) g -> p kb g", p=128),
                id(wk_b): wk.rearrange("(kb p) g -> p kb g", p=128)}
        wo_dst = wo_b.rearrange("p (m d) -> p m d", d=D)
        wo_src2 = wo.rearrange("(m p) d -> p m d", p=128)

        # Waves balanced across the sync/gpsimd queues ONLY -- a DMA issue
        # on the scalar queue would gate the first exp behind the issue
        # stream. Many ~128KB pieces: each dma_start rides one DMA ring, so
        # small pieces across many rings is what parallelizes the head.
        # wave 1: what k0ic0/q0ic0 + scores(p0,ih0,jb0-3) need
        nc.sync.dma_start(mask_t[:], mb[:])
        qq = (nc.sync, nc.gpsimd)
        # k0ic0's very first matmul needs only wk[0][:,0:128] and the token
        # half of xkv[0]: tiny first pieces land in ~5-9us instead of the
        # 12-17us a full 256KB single-ring piece takes, starting the PE (and
        # its HAM warmup) that much earlier
        nc.gpsimd.dma_start(wdst[id(wk_b)][:, 0, 0:128],
                            wsrc[id(wk_b)][:, 0, 0:128])
        nc.sync.dma_start(xkv_dst[:, 0, 0:512], xkv_src[:, 0, 0:512])
        nc.gpsimd.dma_start(wdst[id(wk_b)][:, 0, 128:512],
                            wsrc[id(wk_b)][:, 0, 128:512])
        nc.sync.dma_start(xkv_dst[:, 0, 512:1024], xkv_src[:, 0, 512:1024])
        # xkv 1-2 as halves: fills the 8us head gap after the first matmul
        # (a full-row single-ring piece lands ~19us; halves land ~11-14us)
        for k in (1, 2):
            nc.sync.dma_start(xkv_dst[:, k, 0:512], xkv_src[:, k, 0:512])
            nc.gpsimd.dma_start(xkv_dst[:, k, 512:1024],
                                xkv_src[:, k, 512:1024])
        # full-row x chunks: 2KB lines run at 21.9 GB/s/ring vs 16.4 for 1KB
        for k in range(3, KB):
            qq[k % 2].dma_start(xkv_dst[:, k, :], xkv_src[:, k, :])
        for k in range(1, KB):
            qq[(k + 1) % 2].dma_start(wdst[id(wk_b)][:, k, :],
                                      wsrc[id(wk_b)][:, k, :])
        # wv in wave 1: the v chains run upfront, in the head's otherwise
        # DMA-paced Tensor idle, which empties ~15us of matmuls out of the
        # (Tensor-bound) attention window
        for k in range(KB):
            qq[k % 2].dma_start(wdst[id(wv_b)][:, k, :], wsrc[id(wv_b)][:, k, :])
        for k in range(KB):
            qq[(k + 1) % 2].dma_start(xq_dst[:, k, :], xq_src[:, k, :])
        for k in range(KB):
            qq[k % 2].dma_start(wdst[id(wq_b)][:, k, :],
                                wsrc[id(wq_b)][:, k, :])
        # wave 2
        for m in range(PAIRS):
            qq[m % 2].dma_start(wo_dst[:, m, :], wo_src2[:, m, :])
        # ones blocks via GpSimd memset -- the DMA pattern moved 0.5MB in
        # 128B packets (7 GB/s) through the head's critical DMA window
        for tb in range(TB):
            v2c = v_t[tb].rearrange("p (pb f c) -> p pb f c", f=2, c=128)
            nc.gpsimd.memset(v2c[:, :, :, 0:CH], 1.0)

        # ---------------- v = xkv @ Wv (natural layout) ----------------
        def v_chain(tb):
            ps = psum.tile([128, 512], F32, name="ps_acc", tag="acc", bufs=2)
            for k in range(KB):
                nc.tensor.matmul(
                    ps[:],
                    xkv_t[k][:, tb * 128:(tb + 1) * 128],
                    wv_t[k][:],
                    start=(k == 0),
                    stop=(k == KB - 1),
                )
            v4 = v_t[tb].rearrange("p (pb four c) -> p pb four c",
                                   four=4, c=CH)
            ps3 = ps.rearrange("p (pb two c) -> p pb two c", two=2, c=CH)
            nc.vector.tensor_copy(v4[:, :, 1:2, :], ps3[:, :, 0:1, :])
            nc.vector.tensor_copy(v4[:, :, 3:4, :], ps3[:, :, 1:2, :])

        # (v chains are emitted between k0ic0 and q0ic0 below)

        # ---------------- qT/kT projection chains ----------------
        qT_t = [apool.tile([128, T], BF16, name=f"qT{m}", tag=f"qT{m}")
                for m in range(PAIRS)]
        kT_t = [apool.tile([128, T], BF16, name=f"kT{m}", tag=f"kT{m}")
                for m in range(PAIRS)]

        def proj_chain(p, which, ic):
            src, w_t, dst = ((xq_t, wq_t, qT_t), (xkv_t, wk_t, kT_t))[which]
            csl = slice(ic * 512, (ic + 1) * 512)
            ps = psum.tile([128, 512], F32, name="ps_acc", tag="acc", bufs=2)
            for k in range(KB):
                nc.tensor.matmul(
                    ps[:],
                    w_t[k][:, p * 128:(p + 1) * 128],
                    src[k][:, csl],
                    start=(k == 0),
                    stop=(k == KB - 1),
                )
            nc.vector.tensor_copy(dst[p][:, csl], ps[:])

        # out-projection chain; tb 0..3 read only the ih=0 half of oT, so
        # they can interleave into pair 3's attention
        oq_cycle = [None, None, None]  # filled below (queues)

        ot_full = {}

        def out_chain(tb, ncx, oT_t, trailing=False):
            tsl = slice(tb * 128, (tb + 1) * 128)
            nsl = slice(ncx * 512, (ncx + 1) * 512)
            ps = psum.tile([128, 512], F32, name="ps_acc", tag="acc", bufs=2)
            for m in range(PAIRS):
                nc.tensor.matmul(
                    ps[:],
                    oT_t[m][:, tsl],
                    wo_t[m][:, nsl],
                    start=(m == 0),
                    stop=(m == PAIRS - 1),
                )
            # pair the two 512-col halves into one [128,1024] staging tile
            # so the output DMA moves 2KB lines (f16 512-col would be 1KB)
            if ncx == 0:
                ot_full[tb] = opool.tile([128, 1024], F16, name="ot",
                                         tag="ot", bufs=3)
            ot = ot_full[tb]
            # trailing chains run after the last exp: split the staging
            # copies between ScalarE and VectorE so they drain in parallel
            # (16 serial DVE copies were the tail's critical path). The
            # in-window chains must NOT touch the scalar queue -- its exps.
            if trailing and ncx == 0:
                nc.scalar.copy(ot[:, nsl], ps[:])
            else:
                nc.vector.tensor_copy(ot[:, nsl], ps[:])
            if ncx == 1:
                # two half-DMAs on different queues: a single 256KB piece
                # rides one DMA ring (~12us) and becomes pure tail latency
                for hf in (0, 1):
                    if trailing:
                        q = oq_cycle[(tb * 2 + hf) % 3]
                    else:
                        q = (nc.sync, nc.gpsimd)[(tb * 2 + hf) % 2]
                    q.dma_start(out[tsl, hf * 512:(hf + 1) * 512],
                                ot[:, hf * 512:(hf + 1) * 512])

        # upfront: k0ic0 (earliest-landing inputs), then the v chains (fill
        # the head's DMA-paced Tensor idle), then q0ic0 (latest inputs)
        proj_chain(0, 1, 0)   # k0 ic0
        for tb in range(TB):
            v_chain(tb)
        proj_chain(0, 0, 0)   # q0 ic0

        # ---------------- attention ----------------
        # fill[(p, ih)]: chain thunks run at jb==2 and jb==5 -- the chain a
        # scores MM depends on is always EMITTED earlier, and the Tensor
        # queue executes in order, so this can never deadlock.
        oT_t = [apool.tile([128, T], BF16, name=f"oT{m}", tag=f"oT{m}")
                for m in range(PAIRS)]
        oq_cycle[:] = [nc.scalar, nc.sync, nc.gpsimd]
        # fill[(p, ih)]: {jb: [thunk specs]} run after that jb's scores/exp.
        # The chain a later scores/AV MM depends on is always EMITTED
        # earlier, and the Tensor queue executes in order -> no deadlock.
        fill = {
            (0, 0): {2: [(0, 1, 1)], 5: [(0, 0, 1)]},
            (0, 1): {2: [(1, 0, 0)], 5: [(1, 1, 0)]},
            (1, 0): {2: [(1, 1, 1)], 5: [(1, 0, 1)]},
            (1, 1): {2: [(2, 0, 0)], 5: [(2, 1, 0)]},
            (2, 0): {2: [(2, 1, 1)], 5: [(2, 0, 1)]},
            (2, 1): {2: [(3, 0, 0)], 5: [(3, 1, 0)]},
            (3, 0): {2: [(3, 1, 1)], 5: [(3, 0, 1)]},
            # out-proj tb 0-3 read only the ih0 token half of oT, so they
            # interleave into the last attention block; their DMAs drain
            # during it
            # start at jb2 (the (3,0) norm must finish first); only tb0-1
            # in-window -- tb2-3 run in the trailing region's norm bubble
            (3, 1): {2: [("out", 0, 0)], 3: [("out", 0, 1)],
                     5: [("out", 1, 0)], 6: [("out", 1, 1)]},
        }
        # Blocks run back-to-back with one-step lookahead: the next
        # block's scores(jb0)+exp(jb0) are emitted BEFORE this block's
        # AV-drain + norm, so neither engine idles at block boundaries.
        prev_tail = [None]

        def block(p, ih):
            oT = oT_t[p]
            isl = slice(ih * 512, (ih + 1) * 512)
            po_box = [None]
            pend = []
            slots = fill[(p, ih)]
            for jb in range(TB):
                jsl = slice(jb * 128, (jb + 1) * 128)
                # one [128, 1024] psum tile: bank0 = head 2p, bank1 = 2p+1
                pss = psum.tile([128, 1024], F32, name="ps_s", tag="sc",
                                bufs=2)
                for h in (0, 1):
                    hsl = slice(h * 64, (h + 1) * 64)
                    nc.tensor.matmul(
                        pss[:, h * 512:(h + 1) * 512],
                        kT_t[p][hsl, jsl],
                        qT_t[p][hsl, isl],
                    )
                # one exp for both heads; mask bias is per-partition (= j)
                et = epool.tile([128, 1024], BF16, name="et", tag="et",
                                bufs=6)
                nc.scalar.activation(et[:], pss[:], Exp,
                                     bias=mask_t[:, jb:jb + 1],
                                     scale=SCALE)
                if jb == 0 and prev_tail[0] is not None:
                    prev_tail[0]()
                    prev_tail[0] = None
                pend.append((jb, et))
                if len(pend) > AV_LAG:
                    if po_box[0] is None:
                        po_box[0] = psum.tile([128, 1024], F32, name="po",
                                              tag="po", bufs=1)
                    _av(nc, pend.pop(0), p, po_box[0], v_t)
                for s in slots.get(jb, ()):
                    if s[0] == "v":
                        v_chain(s[1])
                    elif s[0] == "out":
                        out_chain(s[1], s[2], oT_t)
                    else:
                        proj_chain(*s)

            def tail():
                po = po_box[0]
                while pend:
                    _av(nc, pend.pop(0), p, po, v_t)
                # normalize. po bank h = [s_h (64 rows) | o_h (64 rows)]
                # rb = 1/s on VectorE (~51 ULP approx; sums are O(1..1e3));
                # recip inputs MUST be partition-offset-0 APs
                rb_e = npool.tile([CH, 512], F32, name="rb_e", tag="rb",
                                  bufs=4)
                nc.vector.reciprocal_approx_fast(rb_e[:], po[0:CH, 0:512])
                nc.vector.tensor_mul(oT[0:CH, isl], po[CH:128, 0:512],
                                     rb_e[:])
                rb_o = npool.tile([CH, 512], F32, name="rb_o", tag="rb",
                                  bufs=4)
                nc.vector.reciprocal_approx_fast(rb_o[:], po[0:CH, 512:1024])
                nc.vector.tensor_mul(oT[CH:128, isl], po[CH:128, 512:1024],
                                     rb_o[:])

            prev_tail[0] = tail

        for p in range(PAIRS):
            for ih in range(2):
                block(p, ih)
        prev_tail[0]()

        # ---------------- out = oT.T @ Wo (rest) ----------------
        # tb2-3 depend only on ih0 norms (long done): they execute during
        # the 2.8us DVE norm of the last block; tb4+ wait for that norm
        for tb in range(2, TB):
            for ncx in range(2):
                out_chain(tb, ncx, oT_t, trailing=True)


def _av(nc, item, p, po, v_t):
    jb, et = item
    for h in (0, 1):
        # head 2p+h stationary: [ones|v_h] -> out rows [sums | o_h]
        csl = slice(p * VW + h * 128, p * VW + h * 128 + 128)
        nc.tensor.matmul(
            po[:, h * 512:(h + 1) * 512],
            v_t[jb][:, csl],
            et[:, h * 512:(h + 1) * 512],
            start=(jb == 0),
            stop=(jb == TB - 1),
        )


def build_nc():
    nc = bacc.Bacc("TRN2", target_bir_lowering=False, debug=False,
                   num_devices=N_CORES)
    with tile.TileContext(nc) as tc:
        _emit(tc)
    nc.compile()
    return nc


def _get_compiled():
    if "nc" not in _CACHE:
        _CACHE["nc"] = build_nc()
    return _CACHE["nc"]


def make_in_maps(x_q, x_kv, pad_mask):
    ones = np.ones((128, PAIRS * 2 * CH), BF_NP)
    in_maps = []
    for c in range(N_CORES):
        b, g = divmod(c, 2)
        gs = slice(g * G, (g + 1) * G)
        mbias = np.where(pad_mask[b], np.float32(NEG), np.float32(0.0))
        in_maps.append({
            "xqT": np.ascontiguousarray(x_q[b].T).astype(BF_NP),
            "xkvT": np.ascontiguousarray(x_kv[b].T).astype(BF_NP),
            "wq": _W["q"][:, gs].astype(BF_NP),
            "wk": _W["k"][:, gs].astype(BF_NP),
            "wv": _W["v"][:, gs].astype(BF_NP),
            "wo": np.ascontiguousarray(_W["o"][gs, :]).astype(BF_NP),
            "mb": np.ascontiguousarray(mbias.astype(np.float32).reshape(TB, 128).T),
            "vones": ones,
        })
    return in_maps


_W = {}


def kernel(x_q, x_kv, pad_mask, Wq, Wk, Wv, Wo, bo):
    global LAST_RESULTS
    x_q = np.asarray(x_q, dtype=np.float32)
    x_kv = np.asarray(x_kv, dtype=np.float32)
    pad_mask = np.asarray(pad_mask)
    _W["q"] = np.asarray(Wq, dtype=np.float32)
    _W["k"] = np.asarray(Wk, dtype=np.float32)
    _W["v"] = np.asarray(Wv, dtype=np.float32)
    _W["o"] = np.asarray(Wo, dtype=np.float32)
    bo = np.asarray(bo, dtype=np.float32)

    nc = _get_compiled()
    in_maps = make_in_maps(x_q, x_kv, pad_mask)
    res = bass_utils.run_bass_kernel_spmd(nc, in_maps, list(range(N_CORES)))
    LAST_RESULTS = res
    outp = np.zeros((B, T, D), np.float32)
    for b in range(B):
        outp[b] = (res.results[2 * b]["out"].astype(np.float32)
                   + res.results[2 * b + 1]["out"].astype(np.float32))
    outp += bo[None, None, :]
    return outp


# revision 33
# speedup vs baseline: 1.0397x; 1.0146x over previous
"""Multi-head attention, tensor-parallel over heads x data-parallel over batch.

8 NeuronCores: core c handles batch b=c//2, head-group g=c%2 (8 heads, 512 chans).
Each core computes its head-group's attention + partial output projection;
the two partials per batch are summed on the host (row-parallel Wo unshard).

Design (vs the fp32r baseline: 234us -> ~163us):
  - every matmul operand is bf16 (halves DMA + LDWEIGHTS; PSUM stays fp32;
    rel err 7e-3 vs the 2e-2 gate)
  - softmax 1/sum on VectorE via reciprocal_approx_fast, so ScalarE runs
    ONLY Exp (one ACT table set, no FIFO clog from the old ln/exp norm).
    recip silently misreads partition-offset APs, hence the [ones|v]
    stationary packing that puts the sums on partitions 0:64.
  - fp16 output with paired [128,1024] staging (2KB DMA lines), halves the
    output-drain tail
  - only k0/q0 ic0 projection chains run upfront; every other chain (v,
    qk of later pairs, out-proj of the ih0 token half) is interleaved into
    the attention jb-loops via the fill schedule, so TensorE streams
    gap-free behind the ~40us DMA-bandwidth-bound input window
  - one-step block lookahead: the next block's scores+exp are emitted
    before this block's AV-drain + norm (no engine idles at boundaries)
  - input DMA in ~128KB pieces over the sync+gpsimd queues only (each
    dma_start rides ONE ~22GB/s ring; scalar-queue issues would gate the
    first exp); three waves ordered by first use

Per-core dataflow:
  qT/kT = W.T @ xT        [chan, tok] transposed projections
  v     = xkv @ Wv        [tok, chan] natural projection, stored per head
                          pair as [ones|v_even|ones|v_odd] (64 cols each)
  scoresT[j,i] = k.q      row-group packed pairs -> one [128,1024] psum
  expT  = exp(s*scale + maskbias)   one ScalarE op per (pair, ih, jb), bf16 out
  AV:   lhsT = [ones|v_h] (M=128) -> po rows = [sums replicated | o_h]
  norm: rb = recip(sums) on VectorE, oT = po * rb on VectorE (bf16 out)
  out   = oT.T @ Wo       partial output projection
"""

import numpy as np

import concourse.bacc as bacc
import concourse.mybir as mybir
import concourse.tile as tile
from concourse import bass_utils

B = 4
T = 1024          # tokens (N = L)
D = 1024          # model dim
CH = 64           # channels per head
G = 512           # channels per head-group (8 heads)
SCALE = CH ** -0.5
NEG = -30000.0    # mask bias (exp(x + NEG) == 0)
F32 = mybir.dt.float32
F16 = mybir.dt.float16
BF16 = mybir.dt.bfloat16
BF_NP = mybir.dt.np(mybir.dt.bfloat16)
F16_NP = mybir.dt.np(mybir.dt.float16)

N_CORES = 8
KB = 8            # 128-row contraction blocks over D
TB = 8            # 128-token blocks
PAIRS = 4         # head pairs per core
VW = 256          # v-tile columns per head pair: [ones|v_even|ones|v_odd]
AV_LAG = 2        # software-pipeline depth: AV trails scores/exp by this many jb

LAST_RESULTS = None
_CACHE = {}


def _emit(tc):
    nc = tc.nc
    xqT = nc.dram_tensor("xqT", [D, T], BF16, kind="ExternalInput").ap()
    xkvT = nc.dram_tensor("xkvT", [D, T], BF16, kind="ExternalInput").ap()
    wq = nc.dram_tensor("wq", [D, G], BF16, kind="ExternalInput").ap()
    wk = nc.dram_tensor("wk", [D, G], BF16, kind="ExternalInput").ap()
    wv = nc.dram_tensor("wv", [D, G], BF16, kind="ExternalInput").ap()
    wo = nc.dram_tensor("wo", [G, D], BF16, kind="ExternalInput").ap()
    mb = nc.dram_tensor("mb", [128, TB], F32, kind="ExternalInput").ap()
    vones = nc.dram_tensor("vones", [128, PAIRS * 2 * CH], BF16,
                           kind="ExternalInput").ap()
    out = nc.dram_tensor("out", [T, D], F16, kind="ExternalOutput").ap()

    Exp = mybir.ActivationFunctionType.Exp

    with (
        tc.tile_pool(name="wpool", bufs=1) as wpool,
        tc.tile_pool(name="xpool", bufs=1) as xpool,
        tc.tile_pool(name="apool", bufs=1) as apool,
        tc.tile_pool(name="epool", bufs=1) as epool,
        tc.tile_pool(name="npool", bufs=1) as npool,
        tc.tile_pool(name="opool", bufs=4) as opool,
        tc.tile_pool(name="psum", bufs=1, space="PSUM") as psum,
    ):
        # ---------------- input DMA ----------------
        # One dma_start rides ONE queue (~23 GB/s), so transfers are split
        # per 128KB piece and spread over the sync/scalar/gpsimd queues in
        # first-use order. Issue cost is ~0.6-0.9us per dma_start, so each
        # queue carries only what it must before its compute begins.
        mask_t = wpool.tile([128, TB], F32, name="mask_t", tag="mask")

        xkv_b = xpool.tile([128, KB * T], BF16, name="xkv", tag="xkv")
        xq_b = xpool.tile([128, KB * T], BF16, name="xq", tag="xq")
        xkv_src = xkvT.rearrange("(kb p) t -> p kb t", p=128)
        xq_src = xqT.rearrange("(kb p) t -> p kb t", p=128)
        xkv_dst = xkv_b.rearrange("p (kb t) -> p kb t", t=T)
        xq_dst = xq_b.rearrange("p (kb t) -> p kb t", t=T)
        xkv_t = [xkv_b[:, k * T:(k + 1) * T] for k in range(KB)]
        xq_t = [xq_b[:, k * T:(k + 1) * T] for k in range(KB)]

        wv_b = wpool.tile([128, KB * G], BF16, name="wv", tag="wv")
        wq_b = wpool.tile([128, KB * G], BF16, name="wq", tag="wq")
        wk_b = wpool.tile([128, KB * G], BF16, name="wk", tag="wk")
        wv_t = [wv_b[:, k * G:(k + 1) * G] for k in range(KB)]
        wq_t = [wq_b[:, k * G:(k + 1) * G] for k in range(KB)]
        wk_t = [wk_b[:, k * G:(k + 1) * G] for k in range(KB)]
        wo_b = wpool.tile([128, PAIRS * D], BF16, name="wo", tag="wo")
        wo_t = [wo_b[:, m * D:(m + 1) * D] for m in range(PAIRS)]

        v_t = [apool.tile([128, PAIRS * VW], BF16, name=f"v{tb}",
                          tag=f"v{tb}") for tb in range(TB)]
        vo_src = vones.rearrange("p (pb f c) -> p pb f c", f=2, c=CH)
        wdst = {id(wv_b): wv_b.rearrange("p (kb g) -> p kb g", g=G),
                id(wq_b): wq_b.rearrange("p (kb g) -> p kb g", g=G),
                id(wk_b): wk_b.rearrange("p (kb g) -> p kb g", g=G)}
        wsrc = {id(wv_b): wv.rearrange("(kb p) g -> p kb g", p=128),
                id(wq_b): wq.rearrange("(kb p) g -> p kb g", p=128),
                id(wk_b): wk.rearrange("(kb p) g -> p kb g", p=128)}
        wo_dst = wo_b.rearrange("p (m d) -> p m d", d=D)
        wo_src2 = wo.rearrange("(m p) d -> p m d", p=128)

        # Waves balanced across the sync/gpsimd queues ONLY -- a DMA issue
        # on the scalar queue would gate the first exp behind the issue
        # stream. Many ~128KB pieces: each dma_start rides one DMA ring, so
        # small pieces across many rings is what parallelizes the head.
        # wave 1: what k0ic0/q0ic0 + scores(p0,ih0,jb0-3) need
        nc.sync.dma_start(mask_t[:], mb[:])
        qq = (nc.sync, nc.gpsimd)
        # Per-queue FIFO = arrival priority. gpsimd: wk first (k0ic0's
        # stationaries, tiny first piece), then xkv token-half B, then wq.
        # sync: xkv token-half A (k0ic0's moving operands), then wv, then
        # xq half A. The k0ic0 chain streams from ~7us with no 8us stall
        # behind 256KB single-ring pieces.
        nc.gpsimd.dma_start(wdst[id(wk_b)][:, 0, 0:128],
                            wsrc[id(wk_b)][:, 0, 0:128])
        nc.sync.dma_start(xkv_dst[:, 0, 0:512], xkv_src[:, 0, 0:512])
        nc.gpsimd.dma_start(wdst[id(wk_b)][:, 0, 128:512],
                            wsrc[id(wk_b)][:, 0, 128:512])
        for k in range(1, KB):
            nc.gpsimd.dma_start(wdst[id(wk_b)][:, k, :],
                                wsrc[id(wk_b)][:, k, :])
        for k in range(1, KB):
            nc.sync.dma_start(xkv_dst[:, k, 0:512], xkv_src[:, k, 0:512])
        for k in range(KB):
            nc.gpsimd.dma_start(xkv_dst[:, k, 512:1024],
                                xkv_src[:, k, 512:1024])
        for k in range(KB):
            nc.sync.dma_start(wdst[id(wv_b)][:, k, :], wsrc[id(wv_b)][:, k, :])
        for k in range(KB):
            nc.gpsimd.dma_start(wdst[id(wq_b)][:, k, :],
                                wsrc[id(wq_b)][:, k, :])
        for k in range(KB):
            nc.sync.dma_start(xq_dst[:, k, 0:512], xq_src[:, k, 0:512])
        for k in range(KB):
            qq[k % 2].dma_start(xq_dst[:, k, 512:1024],
                                xq_src[:, k, 512:1024])
        for m in range(PAIRS):
            qq[m % 2].dma_start(wo_dst[:, m, :], wo_src2[:, m, :])
        # wave 2
        # ones blocks via GpSimd memset -- the DMA pattern moved 0.5MB in
        # 128B packets (7 GB/s) through the head's critical DMA window
        for tb in range(TB):
            v2c = v_t[tb].rearrange("p (pb f c) -> p pb f c", f=2, c=128)
            nc.gpsimd.memset(v2c[:, :, :, 0:CH], 1.0)

        # ---------------- v = xkv @ Wv (natural layout) ----------------
        def v_chain(tb):
            ps = psum.tile([128, 512], F32, name="ps_acc", tag="acc", bufs=2)
            for k in range(KB):
                nc.tensor.matmul(
                    ps[:],
                    xkv_t[k][:, tb * 128:(tb + 1) * 128],
                    wv_t[k][:],
                    start=(k == 0),
                    stop=(k == KB - 1),
                )
            v4 = v_t[tb].rearrange("p (pb four c) -> p pb four c",
                                   four=4, c=CH)
            ps3 = ps.rearrange("p (pb two c) -> p pb two c", two=2, c=CH)
            nc.vector.tensor_copy(v4[:, :, 1:2, :], ps3[:, :, 0:1, :])
            nc.vector.tensor_copy(v4[:, :, 3:4, :], ps3[:, :, 1:2, :])

        # (v chains are emitted between k0ic0 and q0ic0 below)

        # ---------------- qT/kT projection chains ----------------
        qT_t = [apool.tile([128, T], BF16, name=f"qT{m}", tag=f"qT{m}")
                for m in range(PAIRS)]
        kT_t = [apool.tile([128, T], BF16, name=f"kT{m}", tag=f"kT{m}")
                for m in range(PAIRS)]

        def proj_chain(p, which, ic):
            src, w_t, dst = ((xq_t, wq_t, qT_t), (xkv_t, wk_t, kT_t))[which]
            csl = slice(ic * 512, (ic + 1) * 512)
            ps = psum.tile([128, 512], F32, name="ps_acc", tag="acc", bufs=2)
            for k in range(KB):
                nc.tensor.matmul(
                    ps[:],
                    w_t[k][:, p * 128:(p + 1) * 128],
                    src[k][:, csl],
                    start=(k == 0),
                    stop=(k == KB - 1),
                )
            nc.vector.tensor_copy(dst[p][:, csl], ps[:])

        # out-projection chain; tb 0..3 read only the ih=0 half of oT, so
        # they can interleave into pair 3's attention
        oq_cycle = [None, None, None]  # filled below (queues)

        ot_full = {}

        def out_chain(tb, ncx, oT_t, trailing=False):
            tsl = slice(tb * 128, (tb + 1) * 128)
            nsl = slice(ncx * 512, (ncx + 1) * 512)
            ps = psum.tile([128, 512], F32, name="ps_acc", tag="acc", bufs=2)
            for m in range(PAIRS):
                nc.tensor.matmul(
                    ps[:],
                    oT_t[m][:, tsl],
                    wo_t[m][:, nsl],
                    start=(m == 0),
                    stop=(m == PAIRS - 1),
                )
            # pair the two 512-col halves into one [128,1024] staging tile
            # so the output DMA moves 2KB lines (f16 512-col would be 1KB)
            if ncx == 0:
                ot_full[tb] = opool.tile([128, 1024], F16, name="ot",
                                         tag="ot", bufs=3)
            ot = ot_full[tb]
            # trailing chains run after the last exp: split the staging
            # copies between ScalarE and VectorE so they drain in parallel
            # (16 serial DVE copies were the tail's critical path). The
            # in-window chains must NOT touch the scalar queue -- its exps.
            if trailing and ncx == 0:
                nc.scalar.copy(ot[:, nsl], ps[:])
            else:
                nc.vector.tensor_copy(ot[:, nsl], ps[:])
            if ncx == 1:
                # two half-DMAs on different queues: a single 256KB piece
                # rides one DMA ring (~12us) and becomes pure tail latency
                for hf in (0, 1):
                    if trailing:
                        q = oq_cycle[(tb * 2 + hf) % 3]
                    else:
                        q = (nc.sync, nc.gpsimd)[(tb * 2 + hf) % 2]
                    q.dma_start(out[tsl, hf * 512:(hf + 1) * 512],
                                ot[:, hf * 512:(hf + 1) * 512])

        # upfront: k0ic0 (earliest-landing inputs), then the v chains (fill
        # the head's DMA-paced Tensor idle), then q0ic0 (latest inputs)
        proj_chain(0, 1, 0)   # k0 ic0
        for tb in range(TB):
            v_chain(tb)
        proj_chain(0, 0, 0)   # q0 ic0

        # ---------------- attention ----------------
        # fill[(p, ih)]: chain thunks run at jb==2 and jb==5 -- the chain a
        # scores MM depends on is always EMITTED earlier, and the Tensor
        # queue executes in order, so this can never deadlock.
        oT_t = [apool.tile([128, T], BF16, name=f"oT{m}", tag=f"oT{m}")
                for m in range(PAIRS)]
        oq_cycle[:] = [nc.scalar, nc.sync, nc.gpsimd]
        # fill[(p, ih)]: {jb: [thunk specs]} run after that jb's scores/exp.
        # The chain a later scores/AV MM depends on is always EMITTED
        # earlier, and the Tensor queue executes in order -> no deadlock.
        fill = {
            (0, 0): {2: [(0, 1, 1)], 5: [(0, 0, 1)]},
            (0, 1): {2: [(1, 0, 0)], 5: [(1, 1, 0)]},
            (1, 0): {2: [(1, 1, 1)], 5: [(1, 0, 1)]},
            (1, 1): {2: [(2, 0, 0)], 5: [(2, 1, 0)]},
            (2, 0): {2: [(2, 1, 1)], 5: [(2, 0, 1)]},
            (2, 1): {2: [(3, 0, 0)], 5: [(3, 1, 0)]},
            (3, 0): {2: [(3, 1, 1)], 5: [(3, 0, 1)]},
            # out-proj tb 0-3 read only the ih0 token half of oT, so they
            # interleave into the last attention block; their DMAs drain
            # during it
            # start at jb2 (the (3,0) norm must finish first); only tb0-1
            # in-window -- tb2-3 run in the trailing region's norm bubble
            (3, 1): {2: [("out", 0, 0)], 3: [("out", 0, 1)],
                     5: [("out", 1, 0)], 6: [("out", 1, 1)]},
        }
        # Blocks run back-to-back with one-step lookahead: the next
        # block's scores(jb0)+exp(jb0) are emitted BEFORE this block's
        # AV-drain + norm, so neither engine idles at block boundaries.
        prev_tail = [None]

        def block(p, ih):
            oT = oT_t[p]
            isl = slice(ih * 512, (ih + 1) * 512)
            po_box = [None]
            pend = []
            slots = fill[(p, ih)]
            for jb in range(TB):
                jsl = slice(jb * 128, (jb + 1) * 128)
                # one [128, 1024] psum tile: bank0 = head 2p, bank1 = 2p+1
                pss = psum.tile([128, 1024], F32, name="ps_s", tag="sc",
                                bufs=2)
                for h in (0, 1):
                    hsl = slice(h * 64, (h + 1) * 64)
                    nc.tensor.matmul(
                        pss[:, h * 512:(h + 1) * 512],
                        kT_t[p][hsl, jsl],
                        qT_t[p][hsl, isl],
                    )
                # one exp for both heads; mask bias is per-partition (= j)
                et = epool.tile([128, 1024], BF16, name="et", tag="et",
                                bufs=6)
                nc.scalar.activation(et[:], pss[:], Exp,
                                     bias=mask_t[:, jb:jb + 1],
                                     scale=SCALE)
                if jb == 0 and prev_tail[0] is not None:
                    prev_tail[0]()
                    prev_tail[0] = None
                pend.append((jb, et))
                if len(pend) > AV_LAG:
                    if po_box[0] is None:
                        po_box[0] = psum.tile([128, 1024], F32, name="po",
                                              tag="po", bufs=1)
                    _av(nc, pend.pop(0), p, po_box[0], v_t)
                for s in slots.get(jb, ()):
                    if s[0] == "v":
                        v_chain(s[1])
                    elif s[0] == "out":
                        out_chain(s[1], s[2], oT_t)
                    else:
                        proj_chain(*s)

            def tail():
                po = po_box[0]
                while pend:
                    _av(nc, pend.pop(0), p, po, v_t)
                # normalize. po bank h = [s_h (64 rows) | o_h (64 rows)]
                # rb = 1/s on VectorE (~51 ULP approx; sums are O(1..1e3));
                # recip inputs MUST be partition-offset-0 APs
                rb_e = npool.tile([CH, 512], F32, name="rb_e", tag="rb",
                                  bufs=4)
                nc.vector.reciprocal_approx_fast(rb_e[:], po[0:CH, 0:512])
                nc.vector.tensor_mul(oT[0:CH, isl], po[CH:128, 0:512],
                                     rb_e[:])
                rb_o = npool.tile([CH, 512], F32, name="rb_o", tag="rb",
                                  bufs=4)
                nc.vector.reciprocal_approx_fast(rb_o[:], po[0:CH, 512:1024])
                nc.vector.tensor_mul(oT[CH:128, isl], po[CH:128, 512:1024],
                                     rb_o[:])

            prev_tail[0] = tail

        for p in range(PAIRS):
            for ih in range(2):
                block(p, ih)
        prev_tail[0]()

        # ---------------- out = oT.T @ Wo (rest) ----------------
        # tb2-3 depend only on ih0 norms (long done): they execute during
        # the 2.8us DVE norm of the last block; tb4+ wait for that norm
        for tb in range(2, TB):
            for ncx in range(2):
                out_chain(tb, ncx, oT_t, trailing=True)


def _av(nc, item, p, po, v_t):
    jb, et = item
    for h in (0, 1):
        # head 2p+h stationary: [ones|v_h] -> out rows [sums | o_h]
        csl = slice(p * VW + h * 128, p * VW + h * 128 + 128)
        nc.tensor.matmul(
            po[:, h * 512:(h + 1) * 512],
            v_t[jb][:, csl],
            et[:, h * 512:(h + 1) * 512],
            start=(jb == 0),
            stop=(jb == TB - 1),
        )


def build_nc():
    nc = bacc.Bacc("TRN2", target_bir_lowering=False, debug=False,
                   num_devices=N_CORES)
    with tile.TileContext(nc) as tc:
        _emit(tc)
    nc.compile()
    return nc


def _get_compiled():
    if "nc" not in _CACHE:
        _CACHE["nc"] = build_nc()
    return _CACHE["nc"]


def make_in_maps(x_q, x_kv, pad_mask):
    ones = np.ones((128, PAIRS * 2 * CH), BF_NP)
    in_maps = []
    for c in range(N_CORES):
        b, g = divmod(c, 2)
        gs = slice(g * G, (g + 1) * G)
        mbias = np.where(pad_mask[b], np.float32(NEG), np.float32(0.0))
        in_maps.append({
            "xqT": np.ascontiguousarray(x_q[b].T).astype(BF_NP),
            "xkvT": np.ascontiguousarray(x_kv[b].T).astype(BF_NP),
            "wq": _W["q"][:, gs].astype(BF_NP),
            "wk": _W["k"][:, gs].astype(BF_NP),
            "wv": _W["v"][:, gs].astype(BF_NP),
            "wo": np.ascontiguousarray(_W["o"][gs, :]).astype(BF_NP),
            "mb": np.ascontiguousarray(mbias.astype(np.float32).reshape(TB, 128).T),
            "vones": ones,
        })
    return in_maps


_W = {}


def kernel(x_q, x_kv, pad_mask, Wq, Wk, Wv, Wo, bo):
    global LAST_RESULTS
    x_q = np.asarray(x_q, dtype=np.float32)
    x_kv = np.asarray(x_kv, dtype=np.float32)
    pad_mask = np.asarray(pad_mask)
    _W["q"] = np.asarray(Wq, dtype=np.float32)
    _W["k"] = np.asarray(Wk, dtype=np.float32)
    _W["v"] = np.asarray(Wv, dtype=np.float32)
    _W["o"] = np.asarray(Wo, dtype=np.float32)
    bo = np.asarray(bo, dtype=np.float32)

    nc = _get_compiled()
    in_maps = make_in_maps(x_q, x_kv, pad_mask)
    res = bass_utils.run_bass_kernel_spmd(nc, in_maps, list(range(N_CORES)))
    LAST_RESULTS = res
    outp = np.zeros((B, T, D), np.float32)
    for b in range(B):
        outp[b] = (res.results[2 * b]["out"].astype(np.float32)
                   + res.results[2 * b + 1]["out"].astype(np.float32))
    outp += bo[None, None, :]
    return outp
